# revision 17
# baseline (speedup 1.0000x reference)
"""Trainium2 Bass kernel for NCM/kNN retrieval (nn_NCM_30468497998426).

reference computation:
    mean-center support [C=1000,S=5,D=512] and queries [Q=5000,D=512] by the
    support mean, L2-normalize, sims = einsum('csd,qd->cqs'), max over shots,
    argmax over classes -> [Q] int32.

Sharding: queries split across 8 cores (625 each), support replicated.

Numerics: PE fp16 matmuls run ~4x faster than fp32 (which lowers to a
LOW/HIGH instruction pair). A single fp16 pass cannot separate the closest
class pairs, so sims use an exact 3-term Dekker-style split:
    x = h1 + h2 (+O(2^-22)),  sims = Sh1.q1 + Sh2.q1 + Sh1.q2
Both sides are pre-scaled by 32 (sims scale 1024, argmax-invariant) to keep
the fp16 residuals h2 out of the subnormal range. Error sigma ~2.5e-8 vs a
minimum top-2 class gap of 2.1e-7 in this dataset: exact argmax with margin.

Layout: support padded to 5120 rows (host, zeros) -> 40 tiles of 128 rows;
queries padded to 640 -> [128,128] fp16 stationaries (full PE array + fast
weight load). Support streams twice on the gpsimd/scalar DMA queues (the
sync queue carries queries/outputs); the DVE mean add-tree chases pass 1.
Pass 2 per tile: center (DVE), square+accum / sqrt(x/1024) (ACT), recip
(DVE), scale*inv (ACT, x32), then 4 PE transposes land the tile in one
[128,512] PSUM bank and the PSUM->SBUF copyback IS the fp16 split: ACT
casts h1 into the slab, DVE subtracts for the h2 residual. Slabs hold the
4-5 transposed 128-blocks covering each 500-wide shot-aligned cs chunk, so
chunk j's 12 fp16 matmuls depend only on its own slab; DVE shot-max out of
PSUM, then argmax via max_with_indices.
"""

import numpy as np

import concourse.bacc as bacc
import concourse.mybir as mybir
import concourse.tile as tile
from concourse.alu_op_type import AluOpType
from concourse.bass_utils import run_bass_kernel_spmd

F32 = mybir.dt.float32
F16 = mybir.dt.float16
I32 = mybir.dt.int32
U32 = mybir.dt.uint32
AF = mybir.ActivationFunctionType

C, S, D = 1000, 5, 512
CS = C * S              # 5000 support rows
CSP = 5120              # padded support rows (40 tiles of 128)
Q = 5000
NCORES = 8
QS = Q // NCORES        # 625 queries per core
QSP = 640               # padded queries per core (5 tiles of 128)
P = 128                 # rows per tile
NT = CSP // P           # 40 support tiles
KC = D // 128           # 4 contraction chunks
QT = QSP // P           # 5 query tiles
CSCH = 500              # cs per PSUM chunk (shot-aligned)
NJ = CS // CSCH         # 10 cs chunks
GPC = CSCH // S         # classes per chunk (100)
NG = 4                  # mean-accumulator groups
SCL = 32.0              # fp16 operand pre-scale (sims scale SCL*SCL)


def _slab_blocks(j):
    """Support-tile indices whose transposed 128-blocks cover cs chunk j."""
    b0 = (CSCH * j) // P
    b1 = (CSCH * j + CSCH + P - 1) // P  # exclusive
    return b0, b1


def build():
    nc = bacc.Bacc(None, target_bir_lowering=False)

    sup = nc.declare_dram_parameter("support", [CSP, D], F32, isOutput=False)
    qry = nc.declare_dram_parameter("queries", [QS, D], F32, isOutput=False)
    ident = nc.declare_dram_parameter("ident", [128, 128], F32, isOutput=False)
    ones_col = nc.declare_dram_parameter("ones_col", [128, 1], F32, isOutput=False)
    ones_row = nc.declare_dram_parameter("ones_row", [1, 128], F32, isOutput=False)
    out = nc.declare_dram_parameter("out", [QSP, 1], I32, isOutput=True)

    with tile.TileContext(nc) as tc:
        with (
            tc.tile_pool(name="const", bufs=1) as pconst,
            tc.tile_pool(name="Am", bufs=20) as pam,
            tc.tile_pool(name="Ap", bufs=6) as pap,
            tc.tile_pool(name="qnat", bufs=1) as pq,
            tc.tile_pool(name="q16", bufs=1) as pq16,
            tc.tile_pool(name="acc", bufs=1) as pacc,
            tc.tile_pool(name="stat", bufs=1) as pstat,
            tc.tile_pool(name="slab", bufs=2) as pslab,
            tc.tile_pool(name="scratch", bufs=2) as pscr,
            tc.tile_pool(name="rows", bufs=8) as prows,
            tc.tile_pool(name="best", bufs=1) as pbest,
            tc.tile_pool(name="res", bufs=2) as pres,
            tc.tile_pool(name="trpsum", bufs=1, space="PSUM") as ptr,
            tc.tile_pool(name="mmpsum", bufs=1, space="PSUM") as pmm,
        ):
            id_sb = pconst.tile([128, 128], F32, tag="ident")
            nc.sync.dma_start(id_sb[:], ident[:])
            onec_sb = pconst.tile([128, 1], F32, tag="onec")
            nc.sync.dma_start(onec_sb[:], ones_col[:])
            oner_sb = pconst.tile([1, 128], F32, tag="oner")
            nc.sync.dma_start(oner_sb[:], ones_row[:])

            # ---- pass 1: stream support on gpsimd+scalar queues; queries sync
            with nc.named_scope("load"):
                q_tiles = []
                for i in range(QT):
                    qt_ = pq.tile([P, D], F32, name=f"q{i}", tag=f"q{i}")
                    lo = i * P
                    hi = min((i + 1) * P, QS)
                    if hi - lo < P:
                        nc.vector.memset(qt_[:], 0.0)
                    nc.sync.dma_start(qt_[0:hi - lo, :], qry[lo:hi, :])
                    q_tiles.append(qt_)
                m_tiles = []
                for t in range(NT):
                    mt = pam.tile([P, D], F32, tag="m")
                    eng = nc.sync if t % 2 == 0 else nc.scalar
                    eng.dma_start(mt[:], sup[t * P:(t + 1) * P, :])
                    m_tiles.append(mt)

            # ---- mean add-tree on DVE (chases the DMA stream)
            with nc.named_scope("mean"):
                gacc = []
                for g in range(NG):
                    acc = pacc.tile([P, D], F32, name=f"acc{g}", tag=f"acc{g}")
                    nc.vector.tensor_add(acc[:], m_tiles[g][:],
                                         m_tiles[g + NG][:])
                    gacc.append(acc)
                for r in range(2, NT // NG):
                    for g in range(NG):
                        nc.vector.tensor_add(gacc[g][:], gacc[g][:],
                                             m_tiles[r * NG + g][:])
                nc.vector.tensor_add(gacc[0][:], gacc[0][:], gacc[2][:])
                nc.vector.tensor_add(gacc[1][:], gacc[1][:], gacc[3][:])
                nc.vector.tensor_add(gacc[0][:], gacc[0][:], gacc[1][:])
                mu_ps = ptr.tile([1, D], F32, tag="mu", bufs=1)
                nc.tensor.matmul(mu_ps[:], onec_sb[:], gacc[0][:],
                                 start=True, stop=True)
                mu_sb = pstat.tile([1, D], F32, tag="mu_sb")
                nc.vector.tensor_scalar_mul(mu_sb[:], mu_ps[:], 1.0 / CS)
                mub_ps = ptr.tile([128, D], F32, tag="mub", bufs=1)
                nc.tensor.matmul(mub_ps[:], oner_sb[:], mu_sb[:],
                                 start=True, stop=True)
                mu_b = pstat.tile([128, D], F32, tag="mu_b")
                nc.vector.tensor_copy(mu_b[:], mub_ps[:])

            # ---- query side: center+scale, transpose, split during copyback
            q1_tiles = [pq16.tile([128, QSP], F16, name=f"q1_{k}", tag=f"q1_{k}")
                        for k in range(KC)]
            q2_tiles = [pq16.tile([128, QSP], F16, name=f"q2_{k}", tag=f"q2_{k}")
                        for k in range(KC)]
            with nc.named_scope("qside"):
                for i in range(QT):
                    qt_ = q_tiles[i]
                    nc.vector.tensor_sub(qt_[:], qt_[:], mu_b[:])
                    nc.scalar.activation(qt_[:], qt_[:], AF.Copy, scale=SCL)
                    tq = ptr.tile([128, D], F32, tag="tp", bufs=3)
                    for k in range(KC):
                        ksl = slice(k * 128, (k + 1) * 128)
                        nc.tensor.transpose(tq[:, ksl], qt_[:, ksl], id_sb[:])
                    osl = slice(i * 128, (i + 1) * 128)
                    for k in range(KC):
                        ksl = slice(k * 128, (k + 1) * 128)
                        nc.scalar.copy(q1_tiles[k][:, osl], tq[:, ksl])
                        nc.vector.tensor_sub(q2_tiles[k][:, osl], tq[:, ksl],
                                             q1_tiles[k][:, osl])

            # ---- support pass 2: prep, transpose, split into per-chunk slabs
            best_tiles = [pbest.tile([P, C], F32, name=f"best{i}", tag=f"best{i}")
                          for i in range(QT)]

            # tile t -> chunks j whose slab includes block t
            tile_js = [[] for _ in range(NT)]
            for j in range(NJ):
                b0, b1 = _slab_blocks(j)
                for t in range(b0, b1):
                    tile_js[t].append(j)
            slabs = {}

            def get_slab(j):
                if j not in slabs:
                    b0, b1 = _slab_blocks(j)
                    nb = b1 - b0
                    slabs[j] = (
                        [pslab.tile([128, nb * 128], F16, name=f"s1_{k}_{j}",
                                    tag=f"s1_{k}") for k in range(KC)],
                        [pslab.tile([128, nb * 128], F16, name=f"s2_{k}_{j}",
                                    tag=f"s2_{k}") for k in range(KC)],
                    )
                return slabs[j]

            def do_mm(j):
                s1, s2 = slabs[j]
                b0, _ = _slab_blocks(j)
                off = CSCH * j - b0 * P
                with nc.named_scope(f"mm{j}"):
                    for i in range(QT):
                        ps = pmm.tile([P, CSCH], F32, tag="sims", bufs=3)
                        qsl = slice(i * P, (i + 1) * P)
                        csl = slice(off, off + CSCH)
                        mmseq = []
                        for k in range(KC):
                            mmseq.append((q1_tiles[k][:, qsl], s1[k][:, csl]))
                            mmseq.append((q1_tiles[k][:, qsl], s2[k][:, csl]))
                        for k in range(KC):
                            mmseq.append((q2_tiles[k][:, qsl], s1[k][:, csl]))
                        for n, (lhs, rhs) in enumerate(mmseq):
                            nc.tensor.matmul(ps[:], lhs, rhs,
                                             start=(n == 0),
                                             stop=(n == len(mmseq) - 1))
                        nc.vector.tensor_reduce(
                            out=best_tiles[i][:, j * GPC:(j + 1) * GPC],
                            in_=ps[:].rearrange("p (c s) -> p c s", s=S),
                            axis=mybir.AxisListType.X, op=AluOpType.max,
                        )

            for t in range(NT):
                with nc.named_scope(f"prep{t}"):
                    at = pap.tile([P, D], F32, tag="a")
                    nc.gpsimd.dma_start(at[:], sup[t * P:(t + 1) * P, :])
                    nc.vector.tensor_sub(at[:], at[:], mu_b[:])
                    sq = pscr.tile([P, D], F32, tag="sq")
                    n2 = prows.tile([P, 1], F32, tag="n2")
                    nc.scalar.activation(sq[:], at[:], AF.Square,
                                         accum_out=n2[:])
                    nrm = prows.tile([P, 1], F32, tag="nrm")
                    nc.scalar.activation(nrm[:], n2[:], AF.Sqrt,
                                         scale=1.0 / (SCL * SCL))
                    inv = prows.tile([P, 1], F32, tag="inv")
                    nc.vector.reciprocal(inv[:], nrm[:])
                    nc.vector.tensor_scalar_mul(at[:], at[:], inv[:])
                    tp = ptr.tile([128, D], F32, tag="tp", bufs=3)
                    for k in range(KC):
                        ksl = slice(k * 128, (k + 1) * 128)
                        nc.tensor.transpose(tp[:, ksl], at[:, ksl], id_sb[:])
                    for jn, j in enumerate(tile_js[t]):
                        s1, s2 = get_slab(j)
                        b0, b1 = _slab_blocks(j)
                        osl = slice((t - b0) * 128, (t - b0 + 1) * 128)
                        if jn == 0:
                            for k in range(KC):
                                ksl = slice(k * 128, (k + 1) * 128)
                                nc.scalar.copy(s1[k][:, osl], tp[:, ksl])
                                nc.vector.tensor_sub(s2[k][:, osl], tp[:, ksl],
                                                     s1[k][:, osl])
                        else:
                            p1, p2 = slabs[tile_js[t][0]]
                            pb0 = _slab_blocks(tile_js[t][0])[0]
                            psl = slice((t - pb0) * 128, (t - pb0 + 1) * 128)
                            for k in range(KC):
                                nc.sync.dma_start(s1[k][:, osl], p1[k][:, psl])
                                nc.sync.dma_start(s2[k][:, osl], p2[k][:, psl])
                for j in tile_js[t]:
                    if t == _slab_blocks(j)[1] - 1:
                        do_mm(j)

            # ---- argmax over classes
            with nc.named_scope("argmax"):
                for i in range(QT):
                    mx8 = pres.tile([P, 8], F32, tag="mx8")
                    ix8 = pres.tile([P, 8], U32, tag="ix8")
                    nc.vector.max_with_indices(mx8[:], ix8[:], best_tiles[i][:])
                    ii = pres.tile([P, 1], I32, tag="ii")
                    nc.vector.tensor_copy(ii[:], ix8[:, 0:1])
                    nc.sync.dma_start(out[i * P:(i + 1) * P, :], ii[:])

    nc.finalize()
    return nc


def _host_inputs(support_features, query_features):
    sup = np.zeros((CSP, D), dtype=np.float32)
    sup[:CS] = np.asarray(support_features, dtype=np.float32).reshape(CS, D)
    qf = np.ascontiguousarray(np.asarray(query_features, dtype=np.float32))
    ident = np.eye(128, dtype=np.float32)
    ones_col = np.ones((128, 1), dtype=np.float32)
    ones_row = np.ones((1, 128), dtype=np.float32)
    in_maps = []
    for c in range(NCORES):
        in_maps.append({
            "support": sup,
            "queries": np.ascontiguousarray(qf[c * QS:(c + 1) * QS]),
            "ident": ident,
            "ones_col": ones_col,
            "ones_row": ones_row,
        })
    return in_maps


def run(support_features, query_features, trace=False, **trace_kwargs):
    nc = build()
    in_maps = _host_inputs(support_features, query_features)
    res = run_bass_kernel_spmd(nc, in_maps, list(range(NCORES)),
                               trace=trace, **trace_kwargs)
    outs = [np.asarray(r["out"]).reshape(QSP)[:QS] for r in res.results]
    return np.concatenate(outs).astype(np.int32), res


def kernel(support_features, query_features, use_cosine=None, **_ignored):
    # use_cosine does not change the result: with L2-normalized vectors the
    # euclidean argmin equals the cosine argmax (monotone map), so one kernel
    # serves both branches.
    out, _ = run(support_features, query_features, trace=False)
    return out
